# revision 19
# baseline (speedup 1.0000x reference)
"""Bahdanau attention on 8 Trainium2 NeuronCores — data-parallel over batch.

Per core (4 batches):
  proj[a,s]   = sum_e W_encT[e,a] * encT[e,s]            (PE, bf16, N=512)
  t[a,s]      = tanh(proj + dvec[a,b])                   (ACT, bias fused)
  scores[s]   = sum_a w_v[a] * t[a,s]                    (PE, M=1 matmuls)
  w[s]        = exp(scores)                              (ACT, Z via accum_out)
  ctx[e]     += sum_s encT[e,s] * w[s]                   (DVE bf16 mul + reduce)
  alpha       = w / Z,  context = ctx / Z
softmax(x + b_v) == softmax(x), so b_v drops out of both outputs.
"""

import sys
from contextlib import ExitStack

import ml_dtypes
import numpy as np

if "/opt/trn_rl_repo" not in sys.path:
    sys.path.insert(0, "/opt/trn_rl_repo")

import concourse.bass as bass  # noqa: F401
import concourse.mybir as mybir
import concourse.tile as tile
from concourse import bacc
from concourse.bass_utils import run_bass_kernel_spmd

BF16_NP = ml_dtypes.bfloat16

B, S, E, D, A = 32, 2048, 1024, 1024, 1024
NCORES = 8
BL = B // NCORES          # local batches per core
P = 128                   # partitions
NB = 512                  # s-block (matmul moving dim / PSUM bank)
KC = E // P               # contraction chunks
MC = A // P               # attention-dim chunks
SB = S // NB              # s-blocks per batch
F32 = mybir.dt.float32
BF16 = mybir.dt.bfloat16
AF = mybir.ActivationFunctionType
ALU = mybir.AluOpType
AX = mybir.AxisListType

TRACE = False
LAST_RESULT = None
_CACHE = {}


def _emit(ctx, tc, enc_t, dec_h_t, w_enc_t, w_dec_t, b_sum_pc, w_v_pc,
          ctx_out, alpha_out):
    nc = tc.nc

    const = ctx.enter_context(tc.tile_pool(name="const", bufs=1))
    wenc_pool = ctx.enter_context(tc.tile_pool(name="wenc", bufs=1))
    te_pool = ctx.enter_context(tc.tile_pool(name="te", bufs=5))
    th_pool = ctx.enter_context(tc.tile_pool(name="th", bufs=18))
    wbp_pool = ctx.enter_context(tc.tile_pool(name="wbp", bufs=3))
    dump_pool = ctx.enter_context(tc.tile_pool(name="dump", bufs=2))
    stage_pool = ctx.enter_context(tc.tile_pool(name="stage", bufs=2))
    proj_ps = ctx.enter_context(tc.tile_pool(name="proj_ps", bufs=6, space="PSUM"))
    sc_ps = ctx.enter_context(tc.tile_pool(name="sc_ps", bufs=2, space="PSUM"))

    exp_sb = const.tile([1, BL * S], F32)
    zparts = const.tile([1, BL * SB], F32)
    zsum = const.tile([1, BL], F32)
    zinv = const.tile([1, BL], F32)
    zinv_bc = const.tile([P, BL], F32)
    ctx_parts = const.tile([P, BL * MC * SB], F32)
    dvec = const.tile([P, MC * BL], F32)
    dech = const.tile([P, KC * BL], BF16)
    bsum = const.tile([P, MC], F32)
    wv = const.tile([P, MC], BF16)

    # ---- encoder weights: one tile per m-chunk so unit 0's first chain
    # only waits on 256KB, and later chunks stream in behind te0 ----
    Wm = [wenc_pool.tile([P, KC * P], BF16, tag=f"wm{m}", name=f"wm{m}")
          for m in range(MC)]

    def w_dma(m):
        nc.sync.dma_start(
            Wm[m][:, :].rearrange("p (k j) -> p k j", k=KC),
            w_enc_t[:, m * P:(m + 1) * P].rearrange("(k p) j -> p k j", p=P))

    w_dma(0)

    wd_tiles = []

    def dec_dmas():
        nc.sync.dma_start(dech[:, :].rearrange("p (k b) -> p k b", k=KC),
                          dec_h_t.rearrange("(k p) b -> p k b", p=P))
        nc.sync.dma_start(bsum[:, :], b_sum_pc[:, :])
        nc.sync.dma_start(wv[:, :], w_v_pc[:, :])
        for mh in range(2):
            wd_t = te_pool.tile([P, KC * NB], BF16, tag="te", name=f"wd{mh}")
            nc.sync.dma_start(
                wd_t[:, :].rearrange("p (k a) -> p k a", k=KC),
                w_dec_t[:, mh * NB:(mh + 1) * NB].rearrange("(k p) a -> p k a", p=P))
            wd_tiles.append(wd_t)

    def dec_preamble():
        """dvec[a, b] = W_dec @ h_b + (b_enc + b_dec).

        Matmuls slot into the PE stream mid-way through unit 0's
        projection; dvec's Identity ACTs land before any tanh in the
        ACT queue (tanh waits on dvec via data deps).
        """
        for mh in range(2):
            wd_t = wd_tiles[mh]
            for mi in range(MC // 2):
                m = mh * (MC // 2) + mi
                dps = sc_ps.tile([P, NB], F32, tag="sc")
                for k in range(KC):
                    nc.tensor.matmul(
                        dps[:, :BL],
                        wd_t[:, k * NB + mi * P: k * NB + mi * P + P],
                        dech[:, k * BL:(k + 1) * BL],
                        start=(k == 0),
                        stop=(k == KC - 1),
                    )
                nc.scalar.activation(
                    dvec[:, m * BL:(m + 1) * BL], dps[:, :BL], AF.Identity,
                    bias=bsum[:, m:m + 1],
                )

    warm_src = const.tile([P, NB], BF16)
    nc.vector.memset(warm_src[:, :], 0.0)

    def warmup():
        wp = proj_ps.tile([P, NB], F32, tag="pp", name="warm_ps")
        for i in range(14):
            nc.tensor.matmul(wp[:, :], warm_src[:, :P], warm_src[:, :],
                             start=(i == 0), stop=(i == 13))

    te_tiles = {}

    def ph1(u):
        """DMA the encoder block, projection matmuls, tanh.

        Unit 0 defers its tanhs and last two m-chunks until after the
        decoder preamble (dvec) has been emitted.
        """
        b, sb = divmod(u, SB)
        te = te_pool.tile([P, KC * NB], BF16, tag="te")
        H = KC // 2
        src_ap = enc_t[b, :, sb * NB:(sb + 1) * NB].rearrange(
            "(c k) s -> k c s", k=P)
        if u == 0:
            nc.sync.dma_start(
                te[:, :H * NB].rearrange("k (c s) -> k c s", c=H),
                src_ap[:, :H])
            nc.sync.dma_start(
                te[:, H * NB:].rearrange("k (c s) -> k c s", c=H),
                src_ap[:, H:])
        else:
            nc.sync.dma_start(
                te[:, :].rearrange("k (c s) -> k c s", c=KC), src_ap)
        pps = []
        ths = []

        def proj(m):
            pp = proj_ps.tile([P, NB], F32, tag="pp")
            for k in range(KC):
                nc.tensor.matmul(
                    pp[:, :],
                    Wm[m][:, k * P:(k + 1) * P],
                    te[:, k * NB:(k + 1) * NB],
                    start=(k == 0),
                    stop=(k == KC - 1),
                )
            pps.append(pp)

        def tanh(m):
            th = th_pool.tile([P, NB], BF16, tag="th")
            nc.scalar.activation(
                th[:, :], pps[m][:, :], AF.Tanh,
                bias=dvec[:, m * BL + b: m * BL + b + 1],
            )
            ths.append(th)

        if u == 0:
            dec_dmas()
            for m in range(1, MC):
                w_dma(m)
            warmup()
            for m in range(4):
                proj(m)
            dec_preamble()
            for m in range(4):
                tanh(m)
            for m in range(4, MC):
                proj(m)
                tanh(m)
        else:
            for m in range(MC):
                proj(m)
                tanh(m)
        te_tiles[u] = (te, ths)

    def ph2s(u):
        """Scores matmuls and exp."""
        b, sb = divmod(u, SB)
        te, ths = te_tiles[u]
        sc = sc_ps.tile([1, NB], F32, tag="sc")
        for m in range(MC):
            nc.tensor.matmul(
                sc[:, :],
                wv[:, m:m + 1],
                ths[m][:, :],
                start=(m == 0),
                stop=(m == MC - 1),
            )
        ex = exp_sb[:, b * S + sb * NB: b * S + sb * NB + NB]
        nc.scalar.activation(ex, sc[:, :], AF.Exp,
                             accum_out=zparts[:, u:u + 1])
        exb = wbp_pool.tile([1, NB], BF16, tag="exb")
        nc.scalar.copy(exb[:, :], ex)
        te_tiles[u] = (te, exb)

    def ph2c(u, last=False):
        """Broadcast and context accumulation."""
        te, exb = te_tiles.pop(u)
        wb = wbp_pool.tile([P, NB], BF16, tag="wb")
        nc.gpsimd.partition_broadcast(wb[:, :], exb[:, :])
        dump = dump_pool.tile([P, KC * NB], BF16, tag="dump")
        for c in range(KC):
            nc.vector.tensor_tensor(
                out=dump[:, c * NB:(c + 1) * NB],
                in0=te[:, c * NB:(c + 1) * NB],
                in1=wb[:, :],
                op=ALU.mult,
            )
            if last and c < KC // 2:
                scr = wbp_pool.tile([P, NB], BF16, tag="scr")
                nc.scalar.activation(
                    scr[:, :], dump[:, c * NB:(c + 1) * NB], AF.Copy,
                    accum_out=ctx_parts[:, u * MC + c: u * MC + c + 1])
        H = KC // 2
        if last:
            nc.vector.tensor_reduce(
                out=ctx_parts[:, u * MC + H:(u + 1) * MC],
                in_=dump[:, H * NB:].rearrange("p (c s) -> p c s", c=H),
                axis=AX.X, op=ALU.add,
            )
        else:
            nc.vector.tensor_reduce(
                out=ctx_parts[:, u * MC:(u + 1) * MC],
                in_=dump[:, :].rearrange("p (c s) -> p c s", c=KC),
                axis=AX.X, op=ALU.add,
            )

    def ph2(u):
        ph2s(u)
        ph2c(u)

    def epi_a(b):
        """Normalize and emit alpha for batch b."""
        nc.vector.tensor_reduce(
            out=zsum[:, b:b + 1], in_=zparts[:, b * SB:(b + 1) * SB],
            axis=AX.X, op=ALU.add,
        )
        nc.vector.reciprocal(zinv[:, b:b + 1], zsum[:, b:b + 1])
        alpha_stage = stage_pool.tile([1, S], F32, tag="alpha")
        nc.vector.tensor_scalar_mul(
            alpha_stage[:, :], exp_sb[:, b * S:(b + 1) * S], zinv[:, b:b + 1]
        )
        nc.sync.dma_start(alpha_out[b:b + 1, :], alpha_stage[:, :])
        nc.gpsimd.partition_broadcast(zinv_bc[:, b:b + 1], zinv[:, b:b + 1])

    def epi_c(b):
        """Normalize and emit context for batch b."""
        craw = stage_pool.tile([P, MC], F32, tag="craw")
        nc.vector.tensor_reduce(
            out=craw[:, :],
            in_=ctx_parts[:, b * MC * SB:(b + 1) * MC * SB]
            .rearrange("p (s c) -> p c s", c=MC),
            axis=AX.X, op=ALU.add,
        )
        cs = stage_pool.tile([P, MC], F32, tag="cs")
        nc.vector.tensor_scalar_mul(cs[:, :], craw[:, :], zinv_bc[:, b:b + 1])
        nc.sync.dma_start(
            ctx_out.rearrange("bl (c p) -> bl p c", p=P)[b], cs[:, :]
        )

    # Software-pipelined emission: PE stream stays dense — unit u's scores
    # run behind unit u+1's projection matmuls, hiding ACT latency.
    NU = BL * SB
    ph1(0)
    ph1(1)
    ph2(0)
    for i in range(2, NU - 1):
        ph1(i)
        ph2(i - 1)
        j = i - 2
        if j % SB == SB - 1:
            epi_a(j // SB)
            epi_c(j // SB)
    ph2(NU - 2)
    ph1(NU - 1)
    ph2s(NU - 1)
    epi_a(BL - 1)
    ph2c(NU - 1, last=True)
    epi_c(BL - 1)


def _build():
    nc = bacc.Bacc("TRN2", target_bir_lowering=False, debug=False,
                   enable_asserts=False, num_devices=NCORES)
    enc_t = nc.dram_tensor("enc_t", [BL, E, S], BF16, kind="ExternalInput").ap()
    dec_h_t = nc.dram_tensor("dec_h_t", [D, BL], BF16, kind="ExternalInput").ap()
    w_enc_t = nc.dram_tensor("w_enc_t", [E, A], BF16, kind="ExternalInput").ap()
    w_dec_t = nc.dram_tensor("w_dec_t", [D, A], BF16, kind="ExternalInput").ap()
    b_sum_pc = nc.dram_tensor("b_sum_pc", [P, MC], F32, kind="ExternalInput").ap()
    w_v_pc = nc.dram_tensor("w_v_pc", [P, MC], BF16, kind="ExternalInput").ap()
    ctx_out = nc.dram_tensor("ctx_out", [BL, E], F32, kind="ExternalOutput").ap()
    alpha_out = nc.dram_tensor("alpha_out", [BL, S], F32, kind="ExternalOutput").ap()

    with tile.TileContext(nc) as tc:
        with ExitStack() as ctx:
            _emit(ctx, tc, enc_t, dec_h_t, w_enc_t, w_dec_t, b_sum_pc, w_v_pc,
                  ctx_out, alpha_out)
    nc.compile()
    return nc


def kernel(encoder_out, decoder_hidden, W_enc, b_enc, W_dec, b_dec, W_v, b_v):
    global LAST_RESULT
    if "nc" not in _CACHE:
        _CACHE["nc"] = _build()
    nc = _CACHE["nc"]

    enc = np.asarray(encoder_out, dtype=np.float32)
    dec = np.asarray(decoder_hidden, dtype=np.float32)
    w_enc_t = np.ascontiguousarray(
        np.asarray(W_enc, dtype=np.float32).T.astype(BF16_NP))
    w_dec_t = np.ascontiguousarray(
        np.asarray(W_dec, dtype=np.float32).T.astype(BF16_NP))
    bsum = (np.asarray(b_enc, dtype=np.float32)
            + np.asarray(b_dec, dtype=np.float32))
    b_sum_pc = np.ascontiguousarray(bsum.reshape(MC, P).T)
    w_v_pc = np.ascontiguousarray(
        np.asarray(W_v, dtype=np.float32)[0].reshape(MC, P).T.astype(BF16_NP))

    in_maps = []
    for c in range(NCORES):
        shard = enc[c * BL:(c + 1) * BL]                       # [BL, S, E]
        enc_t = np.ascontiguousarray(
            shard.transpose(0, 2, 1).astype(BF16_NP))           # [BL, E, S]
        dec_h_t = np.ascontiguousarray(
            dec[c * BL:(c + 1) * BL].T.astype(BF16_NP))
        in_maps.append({
            "enc_t": enc_t, "dec_h_t": dec_h_t, "w_enc_t": w_enc_t,
            "w_dec_t": w_dec_t, "b_sum_pc": b_sum_pc, "w_v_pc": w_v_pc,
        })

    res = run_bass_kernel_spmd(nc, in_maps, list(range(NCORES)), trace=TRACE,
                               tmpdir=_CACHE.get("tmpdir"))
    LAST_RESULT = res
    context = np.concatenate([r["ctx_out"] for r in res.results], axis=0)
    alpha = np.concatenate([r["alpha_out"] for r in res.results], axis=0)
    return context, alpha


# revision 20
# speedup vs baseline: 1.0071x; 1.0071x over previous
"""Bahdanau attention on 8 Trainium2 NeuronCores — data-parallel over batch.

Per core (4 batches):
  proj[a,s]   = sum_e W_encT[e,a] * encT[e,s]            (PE, bf16, N=512)
  t[a,s]      = tanh(proj + dvec[a,b])                   (ACT, bias fused)
  scores[s]   = sum_a w_v[a] * t[a,s]                    (PE, M=1 matmuls)
  w[s]        = exp(scores)                              (ACT, Z via accum_out)
  ctx[e]     += sum_s encT[e,s] * w[s]                   (DVE bf16 mul + reduce)
  alpha       = w / Z,  context = ctx / Z
softmax(x + b_v) == softmax(x), so b_v drops out of both outputs.
"""

import sys
from contextlib import ExitStack

import ml_dtypes
import numpy as np

if "/opt/trn_rl_repo" not in sys.path:
    sys.path.insert(0, "/opt/trn_rl_repo")

import concourse.bass as bass  # noqa: F401
import concourse.mybir as mybir
import concourse.tile as tile
from concourse import bacc
from concourse.bass_utils import run_bass_kernel_spmd

BF16_NP = ml_dtypes.bfloat16

B, S, E, D, A = 32, 2048, 1024, 1024, 1024
NCORES = 8
BL = B // NCORES          # local batches per core
P = 128                   # partitions
NB = 512                  # s-block (matmul moving dim / PSUM bank)
KC = E // P               # contraction chunks
MC = A // P               # attention-dim chunks
SB = S // NB              # s-blocks per batch
F32 = mybir.dt.float32
BF16 = mybir.dt.bfloat16
AF = mybir.ActivationFunctionType
ALU = mybir.AluOpType
AX = mybir.AxisListType

TRACE = False
LAST_RESULT = None
_CACHE = {}


def _emit(ctx, tc, enc_t, dec_h_t, w_enc_t, w_dec_t, b_sum_pc, w_v_pc,
          ctx_out, alpha_out):
    nc = tc.nc

    const = ctx.enter_context(tc.tile_pool(name="const", bufs=1))
    wenc_pool = ctx.enter_context(tc.tile_pool(name="wenc", bufs=1))
    te_pool = ctx.enter_context(tc.tile_pool(name="te", bufs=5))
    th_pool = ctx.enter_context(tc.tile_pool(name="th", bufs=18))
    wbp_pool = ctx.enter_context(tc.tile_pool(name="wbp", bufs=3))
    dump_pool = ctx.enter_context(tc.tile_pool(name="dump", bufs=2))
    stage_pool = ctx.enter_context(tc.tile_pool(name="stage", bufs=2))
    proj_ps = ctx.enter_context(tc.tile_pool(name="proj_ps", bufs=6, space="PSUM"))
    sc_ps = ctx.enter_context(tc.tile_pool(name="sc_ps", bufs=2, space="PSUM"))

    exp_sb = const.tile([1, BL * S], F32)
    zparts = const.tile([1, BL * SB], F32)
    zsum = const.tile([1, BL], F32)
    zinv = const.tile([1, BL], F32)
    zinv_bc = const.tile([P, BL], F32)
    ctx_parts = const.tile([P, BL * MC * SB], F32)
    dvec = const.tile([P, MC * BL], F32)
    dech = const.tile([P, KC * BL], BF16)
    bsum = const.tile([P, MC], F32)
    wv = const.tile([P, MC], BF16)

    # ---- encoder weights: one tile per m-chunk so unit 0's first chain
    # only waits on 256KB, and later chunks stream in behind te0 ----
    Wm = [wenc_pool.tile([P, KC * P], BF16, tag=f"wm{m}", name=f"wm{m}")
          for m in range(MC)]

    def w_dma(m):
        nc.sync.dma_start(
            Wm[m][:, :].rearrange("p (k j) -> p k j", k=KC),
            w_enc_t[:, m * P:(m + 1) * P].rearrange("(k p) j -> p k j", p=P))

    w_dma(0)

    wd_tiles = []

    def dec_dmas():
        nc.sync.dma_start(dech[:, :].rearrange("p (k b) -> p k b", k=KC),
                          dec_h_t.rearrange("(k p) b -> p k b", p=P))
        nc.sync.dma_start(bsum[:, :], b_sum_pc[:, :])
        nc.sync.dma_start(wv[:, :], w_v_pc[:, :])
        for mh in range(2):
            wd_t = te_pool.tile([P, KC * NB], BF16, tag="te", name=f"wd{mh}")
            nc.sync.dma_start(
                wd_t[:, :].rearrange("p (k a) -> p k a", k=KC),
                w_dec_t[:, mh * NB:(mh + 1) * NB].rearrange("(k p) a -> p k a", p=P))
            wd_tiles.append(wd_t)

    def dec_m(m):
        """dvec[:, m] = (W_dec @ h + b_enc + b_dec) chunk m — 8 small
        matmuls slotted between unit 0's dense projection chains so the
        PE duty cycle stays high enough for HAM to remain unthrottled."""
        mh, mi = divmod(m, MC // 2)
        wd_t = wd_tiles[mh]
        dps = sc_ps.tile([P, NB], F32, tag="sc")
        for k in range(KC):
            nc.tensor.matmul(
                dps[:, :BL],
                wd_t[:, k * NB + mi * P: k * NB + mi * P + P],
                dech[:, k * BL:(k + 1) * BL],
                start=(k == 0),
                stop=(k == KC - 1),
            )
        nc.scalar.activation(
            dvec[:, m * BL:(m + 1) * BL], dps[:, :BL], AF.Identity,
            bias=bsum[:, m:m + 1],
        )

    warm_src = const.tile([P, NB], BF16)
    nc.vector.memset(warm_src[:, :], 0.0)

    def warmup():
        wp = proj_ps.tile([P, NB], F32, tag="pp", name="warm_ps")
        for i in range(14):
            nc.tensor.matmul(wp[:, :], warm_src[:, :P], warm_src[:, :],
                             start=(i == 0), stop=(i == 13))

    te_tiles = {}

    def ph1(u):
        """DMA the encoder block, projection matmuls, tanh.

        Unit 0 defers its tanhs and last two m-chunks until after the
        decoder preamble (dvec) has been emitted.
        """
        b, sb = divmod(u, SB)
        te = te_pool.tile([P, KC * NB], BF16, tag="te")
        H = KC // 2
        src_ap = enc_t[b, :, sb * NB:(sb + 1) * NB].rearrange(
            "(c k) s -> k c s", k=P)
        if u == 0:
            nc.sync.dma_start(
                te[:, :H * NB].rearrange("k (c s) -> k c s", c=H),
                src_ap[:, :H])
            nc.sync.dma_start(
                te[:, H * NB:].rearrange("k (c s) -> k c s", c=H),
                src_ap[:, H:])
        else:
            nc.sync.dma_start(
                te[:, :].rearrange("k (c s) -> k c s", c=KC), src_ap)
        pps = []
        ths = []

        def proj(m):
            pp = proj_ps.tile([P, NB], F32, tag="pp")
            for k in range(KC):
                nc.tensor.matmul(
                    pp[:, :],
                    Wm[m][:, k * P:(k + 1) * P],
                    te[:, k * NB:(k + 1) * NB],
                    start=(k == 0),
                    stop=(k == KC - 1),
                )
            pps.append(pp)

        def tanh(m):
            th = th_pool.tile([P, NB], BF16, tag="th")
            nc.scalar.activation(
                th[:, :], pps[m][:, :], AF.Tanh,
                bias=dvec[:, m * BL + b: m * BL + b + 1],
            )
            ths.append(th)

        if u == 0:
            dec_dmas()
            for m in range(1, MC):
                w_dma(m)
            warmup()
            for m in range(MC):
                proj(m)
                dec_m(m)
                tanh(m)
        else:
            for m in range(MC):
                proj(m)
                tanh(m)
        te_tiles[u] = (te, ths)

    def ph2s(u):
        """Scores matmuls and exp."""
        b, sb = divmod(u, SB)
        te, ths = te_tiles[u]
        sc = sc_ps.tile([1, NB], F32, tag="sc")
        for m in range(MC):
            nc.tensor.matmul(
                sc[:, :],
                wv[:, m:m + 1],
                ths[m][:, :],
                start=(m == 0),
                stop=(m == MC - 1),
            )
        ex = exp_sb[:, b * S + sb * NB: b * S + sb * NB + NB]
        nc.scalar.activation(ex, sc[:, :], AF.Exp,
                             accum_out=zparts[:, u:u + 1])
        exb = wbp_pool.tile([1, NB], BF16, tag="exb")
        nc.scalar.copy(exb[:, :], ex)
        te_tiles[u] = (te, exb)

    def ph2c(u, last=False):
        """Broadcast and context accumulation."""
        te, exb = te_tiles.pop(u)
        wb = wbp_pool.tile([P, NB], BF16, tag="wb")
        nc.gpsimd.partition_broadcast(wb[:, :], exb[:, :])
        dump = dump_pool.tile([P, KC * NB], BF16, tag="dump")
        for c in range(KC):
            nc.vector.tensor_tensor(
                out=dump[:, c * NB:(c + 1) * NB],
                in0=te[:, c * NB:(c + 1) * NB],
                in1=wb[:, :],
                op=ALU.mult,
            )
            if last and c < KC // 2:
                scr = wbp_pool.tile([P, NB], BF16, tag="scr")
                nc.scalar.activation(
                    scr[:, :], dump[:, c * NB:(c + 1) * NB], AF.Copy,
                    accum_out=ctx_parts[:, u * MC + c: u * MC + c + 1])
        H = KC // 2
        if last:
            nc.vector.tensor_reduce(
                out=ctx_parts[:, u * MC + H:(u + 1) * MC],
                in_=dump[:, H * NB:].rearrange("p (c s) -> p c s", c=H),
                axis=AX.X, op=ALU.add,
            )
        else:
            nc.vector.tensor_reduce(
                out=ctx_parts[:, u * MC:(u + 1) * MC],
                in_=dump[:, :].rearrange("p (c s) -> p c s", c=KC),
                axis=AX.X, op=ALU.add,
            )

    def ph2(u):
        ph2s(u)
        ph2c(u)

    def epi_a(b):
        """Normalize and emit alpha for batch b."""
        nc.vector.tensor_reduce(
            out=zsum[:, b:b + 1], in_=zparts[:, b * SB:(b + 1) * SB],
            axis=AX.X, op=ALU.add,
        )
        nc.vector.reciprocal(zinv[:, b:b + 1], zsum[:, b:b + 1])
        alpha_stage = stage_pool.tile([1, S], F32, tag="alpha")
        nc.vector.tensor_scalar_mul(
            alpha_stage[:, :], exp_sb[:, b * S:(b + 1) * S], zinv[:, b:b + 1]
        )
        nc.sync.dma_start(alpha_out[b:b + 1, :], alpha_stage[:, :])
        nc.gpsimd.partition_broadcast(zinv_bc[:, b:b + 1], zinv[:, b:b + 1])

    def epi_c(b):
        """Normalize and emit context for batch b."""
        craw = stage_pool.tile([P, MC], F32, tag="craw")
        nc.vector.tensor_reduce(
            out=craw[:, :],
            in_=ctx_parts[:, b * MC * SB:(b + 1) * MC * SB]
            .rearrange("p (s c) -> p c s", c=MC),
            axis=AX.X, op=ALU.add,
        )
        cs = stage_pool.tile([P, MC], F32, tag="cs")
        nc.vector.tensor_scalar_mul(cs[:, :], craw[:, :], zinv_bc[:, b:b + 1])
        nc.sync.dma_start(
            ctx_out.rearrange("bl (c p) -> bl p c", p=P)[b], cs[:, :]
        )

    # Software-pipelined emission: PE stream stays dense — unit u's scores
    # run behind unit u+1's projection matmuls, hiding ACT latency.
    NU = BL * SB
    ph1(0)
    ph1(1)
    ph2(0)
    for i in range(2, NU - 1):
        ph1(i)
        ph2(i - 1)
        j = i - 2
        if j % SB == SB - 1:
            epi_a(j // SB)
            epi_c(j // SB)
    ph2(NU - 2)
    ph1(NU - 1)
    ph2s(NU - 1)
    epi_a(BL - 1)
    ph2c(NU - 1, last=True)
    epi_c(BL - 1)


def _build():
    nc = bacc.Bacc("TRN2", target_bir_lowering=False, debug=False,
                   enable_asserts=False, num_devices=NCORES)
    enc_t = nc.dram_tensor("enc_t", [BL, E, S], BF16, kind="ExternalInput").ap()
    dec_h_t = nc.dram_tensor("dec_h_t", [D, BL], BF16, kind="ExternalInput").ap()
    w_enc_t = nc.dram_tensor("w_enc_t", [E, A], BF16, kind="ExternalInput").ap()
    w_dec_t = nc.dram_tensor("w_dec_t", [D, A], BF16, kind="ExternalInput").ap()
    b_sum_pc = nc.dram_tensor("b_sum_pc", [P, MC], F32, kind="ExternalInput").ap()
    w_v_pc = nc.dram_tensor("w_v_pc", [P, MC], BF16, kind="ExternalInput").ap()
    ctx_out = nc.dram_tensor("ctx_out", [BL, E], F32, kind="ExternalOutput").ap()
    alpha_out = nc.dram_tensor("alpha_out", [BL, S], F32, kind="ExternalOutput").ap()

    with tile.TileContext(nc) as tc:
        with ExitStack() as ctx:
            _emit(ctx, tc, enc_t, dec_h_t, w_enc_t, w_dec_t, b_sum_pc, w_v_pc,
                  ctx_out, alpha_out)
    nc.compile()
    return nc


def kernel(encoder_out, decoder_hidden, W_enc, b_enc, W_dec, b_dec, W_v, b_v):
    global LAST_RESULT
    if "nc" not in _CACHE:
        _CACHE["nc"] = _build()
    nc = _CACHE["nc"]

    enc = np.asarray(encoder_out, dtype=np.float32)
    dec = np.asarray(decoder_hidden, dtype=np.float32)
    w_enc_t = np.ascontiguousarray(
        np.asarray(W_enc, dtype=np.float32).T.astype(BF16_NP))
    w_dec_t = np.ascontiguousarray(
        np.asarray(W_dec, dtype=np.float32).T.astype(BF16_NP))
    bsum = (np.asarray(b_enc, dtype=np.float32)
            + np.asarray(b_dec, dtype=np.float32))
    b_sum_pc = np.ascontiguousarray(bsum.reshape(MC, P).T)
    w_v_pc = np.ascontiguousarray(
        np.asarray(W_v, dtype=np.float32)[0].reshape(MC, P).T.astype(BF16_NP))

    in_maps = []
    for c in range(NCORES):
        shard = enc[c * BL:(c + 1) * BL]                       # [BL, S, E]
        enc_t = np.ascontiguousarray(
            shard.transpose(0, 2, 1).astype(BF16_NP))           # [BL, E, S]
        dec_h_t = np.ascontiguousarray(
            dec[c * BL:(c + 1) * BL].T.astype(BF16_NP))
        in_maps.append({
            "enc_t": enc_t, "dec_h_t": dec_h_t, "w_enc_t": w_enc_t,
            "w_dec_t": w_dec_t, "b_sum_pc": b_sum_pc, "w_v_pc": w_v_pc,
        })

    res = run_bass_kernel_spmd(nc, in_maps, list(range(NCORES)), trace=TRACE,
                               tmpdir=_CACHE.get("tmpdir"))
    LAST_RESULT = res
    context = np.concatenate([r["ctx_out"] for r in res.results], axis=0)
    alpha = np.concatenate([r["alpha_out"] for r in res.results], axis=0)
    return context, alpha


# revision 22
# speedup vs baseline: 1.0222x; 1.0150x over previous
"""Bahdanau attention on 8 Trainium2 NeuronCores — data-parallel over batch.

Per core (4 batches):
  proj[a,s]   = sum_e W_encT[e,a] * encT[e,s]            (PE, bf16, N=512)
  t[a,s]      = tanh(proj + dvec[a,b])                   (ACT, bias fused)
  scores[s]   = sum_a w_v[a] * t[a,s]                    (PE, M=1 matmuls)
  w[s]        = exp(scores)                              (ACT, Z via accum_out)
  ctx[e]     += sum_s encT[e,s] * w[s]                   (DVE bf16 mul + reduce)
  alpha       = w / Z,  context = ctx / Z
softmax(x + b_v) == softmax(x), so b_v drops out of both outputs.
"""

import sys
from contextlib import ExitStack

import ml_dtypes
import numpy as np

if "/opt/trn_rl_repo" not in sys.path:
    sys.path.insert(0, "/opt/trn_rl_repo")

import concourse.bass as bass  # noqa: F401
import concourse.mybir as mybir
import concourse.tile as tile
from concourse import bacc
from concourse.bass_utils import run_bass_kernel_spmd

BF16_NP = ml_dtypes.bfloat16

B, S, E, D, A = 32, 2048, 1024, 1024, 1024
NCORES = 8
BL = B // NCORES          # local batches per core
P = 128                   # partitions
NB = 512                  # s-block (matmul moving dim / PSUM bank)
KC = E // P               # contraction chunks
MC = A // P               # attention-dim chunks
SB = S // NB              # s-blocks per batch
F32 = mybir.dt.float32
BF16 = mybir.dt.bfloat16
AF = mybir.ActivationFunctionType
ALU = mybir.AluOpType
AX = mybir.AxisListType

TRACE = False
LAST_RESULT = None
_CACHE = {}


def _emit(ctx, tc, enc_t, dec_h_t, w_enc_t, w_dec_t, b_sum_pc, w_v_pc,
          ctx_out, alpha_out):
    nc = tc.nc

    const = ctx.enter_context(tc.tile_pool(name="const", bufs=1))
    wenc_pool = ctx.enter_context(tc.tile_pool(name="wenc", bufs=1))
    te_pool = ctx.enter_context(tc.tile_pool(name="te", bufs=5))
    th_pool = ctx.enter_context(tc.tile_pool(name="th", bufs=18))
    wbp_pool = ctx.enter_context(tc.tile_pool(name="wbp", bufs=3))
    dump_pool = ctx.enter_context(tc.tile_pool(name="dump", bufs=2))
    stage_pool = ctx.enter_context(tc.tile_pool(name="stage", bufs=2))
    proj_ps = ctx.enter_context(tc.tile_pool(name="proj_ps", bufs=6, space="PSUM"))
    sc_ps = ctx.enter_context(tc.tile_pool(name="sc_ps", bufs=2, space="PSUM"))

    exp_sb = const.tile([1, BL * S], F32)
    zparts = const.tile([1, BL * SB], F32)
    zsum = const.tile([1, BL], F32)
    zinv = const.tile([1, BL], F32)
    zinv_bc = const.tile([P, BL], F32)
    ctx_parts = const.tile([P, BL * MC * SB], F32)
    dvec = const.tile([P, MC * BL], F32)
    dech = const.tile([P, KC * BL], BF16)
    bsum = const.tile([P, MC], F32)
    wv = const.tile([P, MC], BF16)

    # ---- encoder weights: one tile per m-chunk so unit 0's first chain
    # only waits on 256KB, and later chunks stream in behind te0 ----
    Wm = [wenc_pool.tile([P, KC * P], BF16, tag=f"wm{m}", name=f"wm{m}")
          for m in range(MC)]

    def w_dma(m):
        nc.sync.dma_start(
            Wm[m][:, :].rearrange("p (k j) -> p k j", k=KC),
            w_enc_t[:, m * P:(m + 1) * P].rearrange("(k p) j -> p k j", p=P))

    w_dma(0)

    wd_tiles = []

    def dec_dmas():
        nc.sync.dma_start(dech[:, :].rearrange("p (k b) -> p k b", k=KC),
                          dec_h_t.rearrange("(k p) b -> p k b", p=P))
        nc.sync.dma_start(bsum[:, :], b_sum_pc[:, :])
        nc.sync.dma_start(wv[:, :], w_v_pc[:, :])
        nc.vector.tensor_copy(wvf[:, :], wv[:, :])
        for mh in range(2):
            wd_t = te_pool.tile([P, KC * NB], BF16, tag="te", name=f"wd{mh}")
            nc.sync.dma_start(
                wd_t[:, :].rearrange("p (k a) -> p k a", k=KC),
                w_dec_t[:, mh * NB:(mh + 1) * NB].rearrange("(k p) a -> p k a", p=P))
            wd_tiles.append(wd_t)

    def dec_m(m):
        """dvec[:, m] = (W_dec @ h + b_enc + b_dec) chunk m — 8 small
        matmuls slotted between unit 0's dense projection chains so the
        PE duty cycle stays high enough for HAM to remain unthrottled."""
        mh, mi = divmod(m, MC // 2)
        wd_t = wd_tiles[mh]
        dps = sc_ps.tile([P, NB], F32, tag="sc")
        for k in range(KC):
            nc.tensor.matmul(
                dps[:, :BL],
                wd_t[:, k * NB + mi * P: k * NB + mi * P + P],
                dech[:, k * BL:(k + 1) * BL],
                start=(k == 0),
                stop=(k == KC - 1),
            )
        nc.scalar.activation(
            dvec[:, m * BL:(m + 1) * BL], dps[:, :BL], AF.Identity,
            bias=bsum[:, m:m + 1],
        )

    warm_src = const.tile([P, NB], BF16)
    nc.vector.memset(warm_src[:, :], 0.0)
    onecol = const.tile([P, 1], BF16)
    nc.vector.memset(onecol[:, :], 1.0)
    wvf = const.tile([P, MC], F32)

    def warmup():
        wp = proj_ps.tile([P, NB], F32, tag="pp", name="warm_ps")
        for i in range(14):
            nc.tensor.matmul(wp[:, :], warm_src[:, :P], warm_src[:, :],
                             start=(i == 0), stop=(i == 13))

    te_tiles = {}

    def ph1(u):
        """DMA the encoder block, projection matmuls, tanh.

        Unit 0 defers its tanhs and last two m-chunks until after the
        decoder preamble (dvec) has been emitted.
        """
        b, sb = divmod(u, SB)
        te = te_pool.tile([P, KC * NB], BF16, tag="te")
        H = KC // 2
        src_ap = enc_t[b, :, sb * NB:(sb + 1) * NB].rearrange(
            "(c k) s -> k c s", k=P)
        if u == 0:
            nc.sync.dma_start(
                te[:, :H * NB].rearrange("k (c s) -> k c s", c=H),
                src_ap[:, :H])
            nc.sync.dma_start(
                te[:, H * NB:].rearrange("k (c s) -> k c s", c=H),
                src_ap[:, H:])
        else:
            nc.sync.dma_start(
                te[:, :].rearrange("k (c s) -> k c s", c=KC), src_ap)
        pps = []
        ths = []

        def proj(m):
            pp = proj_ps.tile([P, NB], F32, tag="pp")
            for k in range(KC):
                nc.tensor.matmul(
                    pp[:, :],
                    Wm[m][:, k * P:(k + 1) * P],
                    te[:, k * NB:(k + 1) * NB],
                    start=(k == 0),
                    stop=(k == KC - 1),
                )
            pps.append(pp)

        def tanh(m):
            th = th_pool.tile([P, NB], BF16, tag="th")
            nc.scalar.activation(
                th[:, :], pps[m][:, :], AF.Tanh,
                bias=dvec[:, m * BL + b: m * BL + b + 1],
            )
            ths.append(th)

        if u == 0:
            dec_dmas()
            for m in range(1, MC):
                w_dma(m)
            warmup()
            for m in range(MC):
                proj(m)
                dec_m(m)
                tanh(m)
        else:
            for m in range(MC):
                proj(m)
                tanh(m)
        te_tiles[u] = (te, ths)

    def ph2s(u):
        """Scores: m0-3 pre-summed on DVE, m4-7 as wv matmuls."""
        b, sb = divmod(u, SB)
        te, ths = te_tiles[u]
        pacc = wbp_pool.tile([P, NB], BF16, tag="pacc")
        nc.vector.tensor_scalar_mul(pacc[:, :], ths[0][:, :], wvf[:, 0:1])
        for m in range(1, MC // 2):
            ptmp = wbp_pool.tile([P, NB], BF16, tag="ptmp")
            nc.vector.tensor_scalar_mul(ptmp[:, :], ths[m][:, :], wvf[:, m:m + 1])
            nc.vector.tensor_tensor(pacc[:, :], pacc[:, :], ptmp[:, :],
                                    op=ALU.add)
        sc = sc_ps.tile([1, NB], F32, tag="sc")
        nc.tensor.matmul(sc[:, :], onecol[:, :], pacc[:, :],
                         start=True, stop=False)
        for m in range(MC // 2, MC):
            nc.tensor.matmul(
                sc[:, :],
                wv[:, m:m + 1],
                ths[m][:, :],
                start=False,
                stop=(m == MC - 1),
            )
        ex = exp_sb[:, b * S + sb * NB: b * S + sb * NB + NB]
        nc.scalar.activation(ex, sc[:, :], AF.Exp,
                             accum_out=zparts[:, u:u + 1])
        exb = wbp_pool.tile([1, NB], BF16, tag="exb")
        nc.scalar.copy(exb[:, :], ex)
        te_tiles[u] = (te, exb)

    def ph2c(u, last=False):
        """Broadcast and context accumulation."""
        te, exb = te_tiles.pop(u)
        wb = wbp_pool.tile([P, NB], BF16, tag="wb")
        nc.gpsimd.partition_broadcast(wb[:, :], exb[:, :])
        dump = dump_pool.tile([P, KC * NB], BF16, tag="dump")
        for c in range(KC):
            nc.vector.tensor_tensor(
                out=dump[:, c * NB:(c + 1) * NB],
                in0=te[:, c * NB:(c + 1) * NB],
                in1=wb[:, :],
                op=ALU.mult,
            )
            if last and c < KC // 2:
                scr = wbp_pool.tile([P, NB], BF16, tag="scr")
                nc.scalar.activation(
                    scr[:, :], dump[:, c * NB:(c + 1) * NB], AF.Copy,
                    accum_out=ctx_parts[:, u * MC + c: u * MC + c + 1])
        H = KC // 2
        if last:
            nc.vector.tensor_reduce(
                out=ctx_parts[:, u * MC + H:(u + 1) * MC],
                in_=dump[:, H * NB:].rearrange("p (c s) -> p c s", c=H),
                axis=AX.X, op=ALU.add,
            )
        else:
            nc.vector.tensor_reduce(
                out=ctx_parts[:, u * MC:(u + 1) * MC],
                in_=dump[:, :].rearrange("p (c s) -> p c s", c=KC),
                axis=AX.X, op=ALU.add,
            )

    def ph2(u):
        ph2s(u)
        ph2c(u)

    def epi_a(b):
        """Normalize and emit alpha for batch b."""
        nc.vector.tensor_reduce(
            out=zsum[:, b:b + 1], in_=zparts[:, b * SB:(b + 1) * SB],
            axis=AX.X, op=ALU.add,
        )
        nc.vector.reciprocal(zinv[:, b:b + 1], zsum[:, b:b + 1])
        alpha_stage = stage_pool.tile([1, S], F32, tag="alpha")
        nc.vector.tensor_scalar_mul(
            alpha_stage[:, :], exp_sb[:, b * S:(b + 1) * S], zinv[:, b:b + 1]
        )
        nc.sync.dma_start(alpha_out[b:b + 1, :], alpha_stage[:, :])
        nc.gpsimd.partition_broadcast(zinv_bc[:, b:b + 1], zinv[:, b:b + 1])

    def epi_c(b):
        """Normalize and emit context for batch b."""
        craw = stage_pool.tile([P, MC], F32, tag="craw")
        nc.vector.tensor_reduce(
            out=craw[:, :],
            in_=ctx_parts[:, b * MC * SB:(b + 1) * MC * SB]
            .rearrange("p (s c) -> p c s", c=MC),
            axis=AX.X, op=ALU.add,
        )
        cs = stage_pool.tile([P, MC], F32, tag="cs")
        nc.vector.tensor_scalar_mul(cs[:, :], craw[:, :], zinv_bc[:, b:b + 1])
        nc.sync.dma_start(
            ctx_out.rearrange("bl (c p) -> bl p c", p=P)[b], cs[:, :]
        )

    # Software-pipelined emission: PE stream stays dense — unit u's scores
    # run behind unit u+1's projection matmuls, hiding ACT latency.
    NU = BL * SB
    ph1(0)
    ph1(1)
    ph2(0)
    for i in range(2, NU - 1):
        ph1(i)
        ph2(i - 1)
        j = i - 2
        if j % SB == SB - 1:
            epi_a(j // SB)
            epi_c(j // SB)
    ph2(NU - 2)
    ph1(NU - 1)
    ph2s(NU - 1)
    epi_a(BL - 1)
    ph2c(NU - 1, last=True)
    epi_c(BL - 1)


def _build():
    nc = bacc.Bacc("TRN2", target_bir_lowering=False, debug=False,
                   enable_asserts=False, num_devices=NCORES)
    enc_t = nc.dram_tensor("enc_t", [BL, E, S], BF16, kind="ExternalInput").ap()
    dec_h_t = nc.dram_tensor("dec_h_t", [D, BL], BF16, kind="ExternalInput").ap()
    w_enc_t = nc.dram_tensor("w_enc_t", [E, A], BF16, kind="ExternalInput").ap()
    w_dec_t = nc.dram_tensor("w_dec_t", [D, A], BF16, kind="ExternalInput").ap()
    b_sum_pc = nc.dram_tensor("b_sum_pc", [P, MC], F32, kind="ExternalInput").ap()
    w_v_pc = nc.dram_tensor("w_v_pc", [P, MC], BF16, kind="ExternalInput").ap()
    ctx_out = nc.dram_tensor("ctx_out", [BL, E], F32, kind="ExternalOutput").ap()
    alpha_out = nc.dram_tensor("alpha_out", [BL, S], F32, kind="ExternalOutput").ap()

    with tile.TileContext(nc) as tc:
        with ExitStack() as ctx:
            _emit(ctx, tc, enc_t, dec_h_t, w_enc_t, w_dec_t, b_sum_pc, w_v_pc,
                  ctx_out, alpha_out)
    nc.compile()
    return nc


def kernel(encoder_out, decoder_hidden, W_enc, b_enc, W_dec, b_dec, W_v, b_v):
    global LAST_RESULT
    if "nc" not in _CACHE:
        _CACHE["nc"] = _build()
    nc = _CACHE["nc"]

    enc = np.asarray(encoder_out, dtype=np.float32)
    dec = np.asarray(decoder_hidden, dtype=np.float32)
    w_enc_t = np.ascontiguousarray(
        np.asarray(W_enc, dtype=np.float32).T.astype(BF16_NP))
    w_dec_t = np.ascontiguousarray(
        np.asarray(W_dec, dtype=np.float32).T.astype(BF16_NP))
    bsum = (np.asarray(b_enc, dtype=np.float32)
            + np.asarray(b_dec, dtype=np.float32))
    b_sum_pc = np.ascontiguousarray(bsum.reshape(MC, P).T)
    w_v_pc = np.ascontiguousarray(
        np.asarray(W_v, dtype=np.float32)[0].reshape(MC, P).T.astype(BF16_NP))

    in_maps = []
    for c in range(NCORES):
        shard = enc[c * BL:(c + 1) * BL]                       # [BL, S, E]
        enc_t = np.ascontiguousarray(
            shard.transpose(0, 2, 1).astype(BF16_NP))           # [BL, E, S]
        dec_h_t = np.ascontiguousarray(
            dec[c * BL:(c + 1) * BL].T.astype(BF16_NP))
        in_maps.append({
            "enc_t": enc_t, "dec_h_t": dec_h_t, "w_enc_t": w_enc_t,
            "w_dec_t": w_dec_t, "b_sum_pc": b_sum_pc, "w_v_pc": w_v_pc,
        })

    res = run_bass_kernel_spmd(nc, in_maps, list(range(NCORES)), trace=TRACE,
                               tmpdir=_CACHE.get("tmpdir"))
    LAST_RESULT = res
    context = np.concatenate([r["ctx_out"] for r in res.results], axis=0)
    alpha = np.concatenate([r["alpha_out"] for r in res.results], axis=0)
    return context, alpha


# revision 23
# speedup vs baseline: 1.0429x; 1.0203x over previous
"""Bahdanau attention on 8 Trainium2 NeuronCores — data-parallel over batch.

Per core (4 batches):
  proj[a,s]   = sum_e W_encT[e,a] * encT[e,s]            (PE, bf16, N=512)
  t[a,s]      = tanh(proj + dvec[a,b])                   (ACT, bias fused)
  scores[s]   = sum_a w_v[a] * t[a,s]                    (PE, M=1 matmuls)
  w[s]        = exp(scores)                              (ACT, Z via accum_out)
  ctx[e]     += sum_s encT[e,s] * w[s]                   (DVE bf16 mul + reduce)
  alpha       = w / Z,  context = ctx / Z
softmax(x + b_v) == softmax(x), so b_v drops out of both outputs.
"""

import sys
from contextlib import ExitStack

import ml_dtypes
import numpy as np

if "/opt/trn_rl_repo" not in sys.path:
    sys.path.insert(0, "/opt/trn_rl_repo")

import concourse.bass as bass  # noqa: F401
import concourse.mybir as mybir
import concourse.tile as tile
from concourse import bacc
from concourse.bass_utils import run_bass_kernel_spmd

BF16_NP = ml_dtypes.bfloat16

B, S, E, D, A = 32, 2048, 1024, 1024, 1024
NCORES = 8
BL = B // NCORES          # local batches per core
P = 128                   # partitions
NB = 512                  # s-block (matmul moving dim / PSUM bank)
KC = E // P               # contraction chunks
MC = A // P               # attention-dim chunks
SB = S // NB              # s-blocks per batch
F32 = mybir.dt.float32
BF16 = mybir.dt.bfloat16
AF = mybir.ActivationFunctionType
ALU = mybir.AluOpType
AX = mybir.AxisListType

TRACE = False
LAST_RESULT = None
_CACHE = {}


def _emit(ctx, tc, enc_t, dec_h_t, w_enc_t, w_dec_t, b_sum_pc, w_v_pc,
          ctx_out, alpha_out):
    nc = tc.nc

    const = ctx.enter_context(tc.tile_pool(name="const", bufs=1))
    wenc_pool = ctx.enter_context(tc.tile_pool(name="wenc", bufs=1))
    te_pool = ctx.enter_context(tc.tile_pool(name="te", bufs=5))
    th_pool = ctx.enter_context(tc.tile_pool(name="th", bufs=18))
    wbp_pool = ctx.enter_context(tc.tile_pool(name="wbp", bufs=3))
    dump_pool = ctx.enter_context(tc.tile_pool(name="dump", bufs=2))
    stage_pool = ctx.enter_context(tc.tile_pool(name="stage", bufs=2))
    proj_ps = ctx.enter_context(tc.tile_pool(name="proj_ps", bufs=6, space="PSUM"))
    sc_ps = ctx.enter_context(tc.tile_pool(name="sc_ps", bufs=2, space="PSUM"))

    exp_sb = const.tile([1, BL * S], F32)
    zparts = const.tile([1, BL * SB], F32)
    zsum = const.tile([1, BL], F32)
    zinv = const.tile([1, BL], F32)
    zinv_bc = const.tile([P, BL], F32)
    ctx_parts = const.tile([P, BL * MC * SB], F32)
    dvec = const.tile([P, MC * BL], F32)
    dech = const.tile([P, KC * BL], BF16)
    bsum = const.tile([P, MC], F32)
    wv = const.tile([P, MC], BF16)

    # ---- encoder weights: one tile per m-chunk so unit 0's first chain
    # only waits on 256KB, and later chunks stream in behind te0 ----
    Wm = [wenc_pool.tile([P, KC * P], BF16, tag=f"wm{m}", name=f"wm{m}")
          for m in range(MC)]

    def w_dma(m):
        nc.sync.dma_start(
            Wm[m][:, :].rearrange("p (k j) -> p k j", k=KC),
            w_enc_t[:, m * P:(m + 1) * P].rearrange("(k p) j -> p k j", p=P))

    w_dma(0)

    wd_tiles = []

    def dec_dmas():
        nc.sync.dma_start(dech[:, :].rearrange("p (k b) -> p k b", k=KC),
                          dec_h_t.rearrange("(k p) b -> p k b", p=P))
        nc.sync.dma_start(bsum[:, :], b_sum_pc[:, :])
        nc.sync.dma_start(wv[:, :], w_v_pc[:, :])
        nc.vector.tensor_copy(wvf[:, :], wv[:, :])
        for mh in range(2):
            wd_t = te_pool.tile([P, KC * NB], BF16, tag="te", name=f"wd{mh}")
            nc.sync.dma_start(
                wd_t[:, :].rearrange("p (k a) -> p k a", k=KC),
                w_dec_t[:, mh * NB:(mh + 1) * NB].rearrange("(k p) a -> p k a", p=P))
            wd_tiles.append(wd_t)

    def dec_m(m):
        """dvec[:, m] = (W_dec @ h + b_enc + b_dec) chunk m — 8 small
        matmuls slotted between unit 0's dense projection chains so the
        PE duty cycle stays high enough for HAM to remain unthrottled."""
        mh, mi = divmod(m, MC // 2)
        wd_t = wd_tiles[mh]
        dps = sc_ps.tile([P, NB], F32, tag="sc")
        for k in range(KC):
            nc.tensor.matmul(
                dps[:, :BL],
                wd_t[:, k * NB + mi * P: k * NB + mi * P + P],
                dech[:, k * BL:(k + 1) * BL],
                start=(k == 0),
                stop=(k == KC - 1),
            )
        nc.scalar.activation(
            dvec[:, m * BL:(m + 1) * BL], dps[:, :BL], AF.Identity,
            bias=bsum[:, m:m + 1],
        )

    warm_src = const.tile([P, NB], BF16)
    nc.vector.memset(warm_src[:, :], 0.0)
    onecol = const.tile([P, 1], BF16)
    nc.vector.memset(onecol[:, :], 1.0)
    wvf = const.tile([P, MC], F32)

    def warmup():
        wp = proj_ps.tile([P, NB], F32, tag="pp", name="warm_ps")
        for i in range(14):
            nc.tensor.matmul(wp[:, :], warm_src[:, :P], warm_src[:, :],
                             start=(i == 0), stop=(i == 13))

    te_tiles = {}

    def ph1(u):
        """DMA the encoder block, projection matmuls, tanh.

        Unit 0 defers its tanhs and last two m-chunks until after the
        decoder preamble (dvec) has been emitted.
        """
        b, sb = divmod(u, SB)
        te = te_pool.tile([P, KC * NB], BF16, tag="te")
        H = KC // 2
        src_ap = enc_t[b, :, sb * NB:(sb + 1) * NB].rearrange(
            "(c k) s -> k c s", k=P)
        if u == 0:
            nc.sync.dma_start(
                te[:, :H * NB].rearrange("k (c s) -> k c s", c=H),
                src_ap[:, :H])
            nc.sync.dma_start(
                te[:, H * NB:].rearrange("k (c s) -> k c s", c=H),
                src_ap[:, H:])
        else:
            nc.sync.dma_start(
                te[:, :].rearrange("k (c s) -> k c s", c=KC), src_ap)
        pps = []
        ths = []

        def proj(m):
            pp = proj_ps.tile([P, NB], F32, tag="pp")
            for k in range(KC):
                nc.tensor.matmul(
                    pp[:, :],
                    Wm[m][:, k * P:(k + 1) * P],
                    te[:, k * NB:(k + 1) * NB],
                    start=(k == 0),
                    stop=(k == KC - 1),
                )
            pps.append(pp)

        def tanh(m):
            th = th_pool.tile([P, NB], BF16, tag="th")
            nc.scalar.activation(
                th[:, :], pps[m][:, :], AF.Tanh,
                bias=dvec[:, m * BL + b: m * BL + b + 1],
            )
            ths.append(th)

        if u == 0:
            dec_dmas()
            for m in range(1, MC):
                w_dma(m)
            warmup()
            for m in range(MC):
                proj(m)
                dec_m(m)
                tanh(m)
        else:
            for m in range(MC):
                proj(m)
                tanh(m)
        te_tiles[u] = (te, ths)

    def ph2s(u):
        """Scores: m0-3 pre-summed on DVE, m4-7 as wv matmuls."""
        b, sb = divmod(u, SB)
        te, ths = te_tiles[u]
        NDVE = 6
        pacc = wbp_pool.tile([P, NB], BF16, tag="pacc")
        nc.vector.tensor_scalar_mul(pacc[:, :], ths[0][:, :], wvf[:, 0:1])
        for m in range(1, NDVE):
            ptmp = wbp_pool.tile([P, NB], BF16, tag="ptmp")
            nc.vector.tensor_scalar_mul(ptmp[:, :], ths[m][:, :], wvf[:, m:m + 1])
            nc.vector.tensor_tensor(pacc[:, :], pacc[:, :], ptmp[:, :],
                                    op=ALU.add)
        sc = sc_ps.tile([1, NB], F32, tag="sc")
        nc.tensor.matmul(sc[:, :], onecol[:, :], pacc[:, :],
                         start=True, stop=False)
        for m in range(NDVE, MC):
            nc.tensor.matmul(
                sc[:, :],
                wv[:, m:m + 1],
                ths[m][:, :],
                start=False,
                stop=(m == MC - 1),
            )
        ex = exp_sb[:, b * S + sb * NB: b * S + sb * NB + NB]
        nc.scalar.activation(ex, sc[:, :], AF.Exp,
                             accum_out=zparts[:, u:u + 1])
        exb = wbp_pool.tile([1, NB], BF16, tag="exb")
        nc.scalar.copy(exb[:, :], ex)
        te_tiles[u] = (te, exb)

    def ph2c(u, last=False):
        """Broadcast and context accumulation."""
        te, exb = te_tiles.pop(u)
        wb = wbp_pool.tile([P, NB], BF16, tag="wb")
        nc.gpsimd.partition_broadcast(wb[:, :], exb[:, :])
        dump = dump_pool.tile([P, KC * NB], BF16, tag="dump")
        for c in range(KC):
            nc.vector.tensor_tensor(
                out=dump[:, c * NB:(c + 1) * NB],
                in0=te[:, c * NB:(c + 1) * NB],
                in1=wb[:, :],
                op=ALU.mult,
            )
            if last and c < KC // 2:
                scr = wbp_pool.tile([P, NB], BF16, tag="scr")
                nc.scalar.activation(
                    scr[:, :], dump[:, c * NB:(c + 1) * NB], AF.Copy,
                    accum_out=ctx_parts[:, u * MC + c: u * MC + c + 1])
        H = KC // 2
        if last:
            nc.vector.tensor_reduce(
                out=ctx_parts[:, u * MC + H:(u + 1) * MC],
                in_=dump[:, H * NB:].rearrange("p (c s) -> p c s", c=H),
                axis=AX.X, op=ALU.add,
            )
        else:
            nc.vector.tensor_reduce(
                out=ctx_parts[:, u * MC:(u + 1) * MC],
                in_=dump[:, :].rearrange("p (c s) -> p c s", c=KC),
                axis=AX.X, op=ALU.add,
            )

    def ph2(u):
        ph2s(u)
        ph2c(u)

    def epi_a(b):
        """Normalize and emit alpha for batch b."""
        nc.vector.tensor_reduce(
            out=zsum[:, b:b + 1], in_=zparts[:, b * SB:(b + 1) * SB],
            axis=AX.X, op=ALU.add,
        )
        nc.vector.reciprocal(zinv[:, b:b + 1], zsum[:, b:b + 1])
        alpha_stage = stage_pool.tile([1, S], F32, tag="alpha")
        nc.vector.tensor_scalar_mul(
            alpha_stage[:, :], exp_sb[:, b * S:(b + 1) * S], zinv[:, b:b + 1]
        )
        nc.sync.dma_start(alpha_out[b:b + 1, :], alpha_stage[:, :])
        nc.gpsimd.partition_broadcast(zinv_bc[:, b:b + 1], zinv[:, b:b + 1])

    def epi_c(b):
        """Normalize and emit context for batch b."""
        craw = stage_pool.tile([P, MC], F32, tag="craw")
        nc.vector.tensor_reduce(
            out=craw[:, :],
            in_=ctx_parts[:, b * MC * SB:(b + 1) * MC * SB]
            .rearrange("p (s c) -> p c s", c=MC),
            axis=AX.X, op=ALU.add,
        )
        cs = stage_pool.tile([P, MC], F32, tag="cs")
        nc.vector.tensor_scalar_mul(cs[:, :], craw[:, :], zinv_bc[:, b:b + 1])
        nc.sync.dma_start(
            ctx_out.rearrange("bl (c p) -> bl p c", p=P)[b], cs[:, :]
        )

    # Software-pipelined emission: PE stream stays dense — unit u's scores
    # run behind unit u+1's projection matmuls, hiding ACT latency.
    NU = BL * SB
    ph1(0)
    ph1(1)
    ph2(0)
    for i in range(2, NU - 1):
        ph1(i)
        ph2(i - 1)
        j = i - 2
        if j % SB == SB - 1:
            epi_a(j // SB)
            epi_c(j // SB)
    ph2(NU - 2)
    ph1(NU - 1)
    ph2s(NU - 1)
    epi_a(BL - 1)
    ph2c(NU - 1, last=True)
    epi_c(BL - 1)


def _build():
    nc = bacc.Bacc("TRN2", target_bir_lowering=False, debug=False,
                   enable_asserts=False, num_devices=NCORES)
    enc_t = nc.dram_tensor("enc_t", [BL, E, S], BF16, kind="ExternalInput").ap()
    dec_h_t = nc.dram_tensor("dec_h_t", [D, BL], BF16, kind="ExternalInput").ap()
    w_enc_t = nc.dram_tensor("w_enc_t", [E, A], BF16, kind="ExternalInput").ap()
    w_dec_t = nc.dram_tensor("w_dec_t", [D, A], BF16, kind="ExternalInput").ap()
    b_sum_pc = nc.dram_tensor("b_sum_pc", [P, MC], F32, kind="ExternalInput").ap()
    w_v_pc = nc.dram_tensor("w_v_pc", [P, MC], BF16, kind="ExternalInput").ap()
    ctx_out = nc.dram_tensor("ctx_out", [BL, E], F32, kind="ExternalOutput").ap()
    alpha_out = nc.dram_tensor("alpha_out", [BL, S], F32, kind="ExternalOutput").ap()

    with tile.TileContext(nc) as tc:
        with ExitStack() as ctx:
            _emit(ctx, tc, enc_t, dec_h_t, w_enc_t, w_dec_t, b_sum_pc, w_v_pc,
                  ctx_out, alpha_out)
    nc.compile()
    return nc


def kernel(encoder_out, decoder_hidden, W_enc, b_enc, W_dec, b_dec, W_v, b_v):
    global LAST_RESULT
    if "nc" not in _CACHE:
        _CACHE["nc"] = _build()
    nc = _CACHE["nc"]

    enc = np.asarray(encoder_out, dtype=np.float32)
    dec = np.asarray(decoder_hidden, dtype=np.float32)
    w_enc_t = np.ascontiguousarray(
        np.asarray(W_enc, dtype=np.float32).T.astype(BF16_NP))
    w_dec_t = np.ascontiguousarray(
        np.asarray(W_dec, dtype=np.float32).T.astype(BF16_NP))
    bsum = (np.asarray(b_enc, dtype=np.float32)
            + np.asarray(b_dec, dtype=np.float32))
    b_sum_pc = np.ascontiguousarray(bsum.reshape(MC, P).T)
    w_v_pc = np.ascontiguousarray(
        np.asarray(W_v, dtype=np.float32)[0].reshape(MC, P).T.astype(BF16_NP))

    in_maps = []
    for c in range(NCORES):
        shard = enc[c * BL:(c + 1) * BL]                       # [BL, S, E]
        enc_t = np.ascontiguousarray(
            shard.transpose(0, 2, 1).astype(BF16_NP))           # [BL, E, S]
        dec_h_t = np.ascontiguousarray(
            dec[c * BL:(c + 1) * BL].T.astype(BF16_NP))
        in_maps.append({
            "enc_t": enc_t, "dec_h_t": dec_h_t, "w_enc_t": w_enc_t,
            "w_dec_t": w_dec_t, "b_sum_pc": b_sum_pc, "w_v_pc": w_v_pc,
        })

    res = run_bass_kernel_spmd(nc, in_maps, list(range(NCORES)), trace=TRACE,
                               tmpdir=_CACHE.get("tmpdir"))
    LAST_RESULT = res
    context = np.concatenate([r["ctx_out"] for r in res.results], axis=0)
    alpha = np.concatenate([r["alpha_out"] for r in res.results], axis=0)
    return context, alpha
